# revision 61
# baseline (speedup 1.0000x reference)
"""Behler symmetry functions (set-51: 8 G2 + 43 G4) on 8 Trainium2 cores.

Sharding: data-parallel over atoms. Each core handles 250 atoms (2 tiles of
<=128 atoms on partitions). Per tile: gather the 32 neighbor positions per
atom from a replicated pos table in DRAM (indirect DMA), then run the pair
stage on a cyclic diagonal packing: pair (j, (j+d) mod 32) for d=1..16, 512
slots; the d=16 block double-counts each unordered pair so its weight is
halved. G4 columns come from fused multiply-reduce lanes (DVE
tensor_tensor_reduce, accum_out = one output column per instruction):

  s'   = rsq_j + rsq_k - dot      (so rij^2+rik^2+rjk^2 = 2 s')
  rjk^2 = s' - dot
  cos  = dot * invr_j * invr_k
  w    = fcj*fck*2fc_jk;  E_eta = exp(-2 eta s')
  zeta<=2 cols: binomial recombination of moments M_m = sum c^m w E
  zeta=4 cols:  direct lanes sum ((1 +- c)/2)^4 w E  (cancellation-free)
  zeta=16 col:  direct lane with ((1+c)/2)^16 via ACT square chain
"""

import sys

sys.path.insert(0, "/opt/trn_rl_repo")

import numpy as np

import concourse.bass as bass
import concourse.mybir as mybir
from concourse.bass import AP, IndirectOffsetOnAxis
from concourse.tile import TileContext
from concourse.bass_utils import run_bass_kernel_spmd

AF = mybir.ActivationFunctionType
ALU = mybir.AluOpType
DT = mybir.dt

N_ATOMS = 2000
K = 32
K2 = 2 * K
N_CORES = 8
APC = N_ATOMS // N_CORES          # atoms per core (250)
TILES = 2                         # partition tiles per core (128 + 122)
P = 128
SENT = N_ATOMS                    # sentinel table row (far away)
RCUT = 8.0
ND = 16                           # cyclic diagonals
NP2 = ND * K                      # pair slots (512)

G2_ETA = [0.0036, 0.036, 0.071, 0.125, 0.214, 0.357, 0.714, 1.428]
ETAS7 = [0.0001, 0.003, 0.008, 0.015, 0.025, 0.045, 0.08]
NM = 3                            # moments m = 0..2 (zeta <= 2)

# recombination: col(e, ci) = sum_m CO[ci][m] * M_m(e), ci=(lam,zeta):
# (-1,1),(1,1),(-1,2),(1,2)
CO4 = [
    [1 / 2, -1 / 2, 0],
    [1 / 2, 1 / 2, 0],
    [1 / 4, -2 / 4, 1 / 4],
    [1 / 4, 2 / 4, 1 / 4],
]

# lanes offloaded to Pool-mult + ACT-accumulate per tile; tile 1 gets more
# because the Pool engine is free once the tile-1 gathers drain.
POOL_LANES_PER_TILE = [0, 12]
# lanes run as DVE 2x-mode mult (no accum, 327ns) + ACT accumulate; bounded
# by ACT spare capacity
ACT_LANES_PER_TILE = [0, 0]

MAX_WAITS_PER_INST = 1


def _split_excess_waits(nc):
    """This toolchain rejects instructions carrying more than ~2 sem waits.
    Move excess waits onto NoOp carriers spliced before, same engine."""
    for fn in nc.m.functions:
        for bb in fn.blocks:
            new_list = []
            changed = False
            for inst in bb.instructions:
                si = inst.sync_info
                if si is not None and len(si.on_wait) > MAX_WAITS_PER_INST:
                    waits = list(si.on_wait)
                    extra = waits[:-MAX_WAITS_PER_INST]
                    keep = waits[-MAX_WAITS_PER_INST:]
                    for i in range(0, len(extra), MAX_WAITS_PER_INST):
                        nop = mybir.InstNoOp(
                            name=f"WS-{nc.next_id()}",
                            engine=inst.engine,
                            sync_info=mybir.SyncInfo(
                                on_wait=extra[i : i + MAX_WAITS_PER_INST], on_update=[]
                            ),
                            bass_nofuse=True,
                        )
                        nc.register_instruction(nop, overwrite=True)
                        new_list.append(nop)
                    inst.sync_info = mybir.SyncInfo(
                        on_wait=keep, on_update=list(si.on_update)
                    )
                    changed = True
                new_list.append(inst)
            if changed:
                bb.instructions = new_list


def _v(tile_ap, offset_ap, dims):
    """AP view of a tile: dims = [[step, count], ...] free dims."""
    return AP(offset_ap.tensor, offset_ap.offset, [offset_ap.ap[0]] + dims)


def build_nc():
    nc = bass.Bass()

    def register_const(value, dtype=DT.float32):
        t = nc.alloc_sbuf_tensor(f"const-{dtype.name}-{value}", [P, 1], dtype)
        nc.gpsimd.memset(t.ap(), value)
        nc.const_aps.aps[(dtype, value)] = t.ap()

    register_const(float(np.pi / 2))
    register_const(float(-np.pi / 2))
    register_const(0.5)
    nc.all_engine_barrier()

    tbl = nc.declare_dram_parameter("pos_tbl", [N_ATOMS + 1, 3], DT.float32, isOutput=False)
    idx_in = nc.declare_dram_parameter("idx", [P, TILES, K], DT.int32, isOutput=False)
    own_in = nc.declare_dram_parameter("own", [P, TILES, 3], DT.float32, isOutput=False)
    co_in = nc.declare_dram_parameter("co", [P, NM, 4], DT.float32, isOutput=False)
    out_d = nc.declare_dram_parameter("out", [P, TILES, 51], DT.float32, isOutput=True)

    with TileContext(nc) as tc:
        with (
            tc.tile_pool(name="io", bufs=1) as iop,
            tc.tile_pool(name="work", bufs=2) as wp,
            tc.tile_pool(name="scr", bufs=2) as scp,
        ):
            idx_t = iop.tile([P, TILES, K], DT.int32)
            nc.sync.dma_start(out=idx_t[:], in_=idx_in[:])
            own_t = iop.tile([P, TILES, 3], DT.float32)
            nc.sync.dma_start(out=own_t[:], in_=own_in[:])
            co_t = iop.tile([P, NM, 4], DT.float32)
            nc.sync.dma_start(out=co_t[:], in_=co_in[:])

            Gs = []
            for t in range(TILES):
                G = wp.tile([P, K, 3], DT.float32, tag="G", name=f"G{t}")
                for k in range(K):
                    nc.gpsimd.indirect_dma_start(
                        out=G[:, k],
                        out_offset=None,
                        in_=tbl[:],
                        in_offset=IndirectOffsetOnAxis(
                            ap=idx_t[:, t, k : k + 1], axis=0
                        ),
                    )
                Gs.append(G)
            for t in range(TILES):
                # floor tile t's compute in the scheduler's simulated clock so
                # tile 0's lane block isn't dammed behind tile 1's gather
                # waits; tile 1's small neighbor stage gets an earlier floor
                # so it can fill tile 0's bubbles
                _tile_body(nc, tc, wp, scp, Gs[t], idx_t, own_t, co_t, out_d, t,
                           nb_floor=0.055 * t, pair_floor=0.2 * t)

    _split_excess_waits(nc)
    return nc


def _tile_body(nc, tc, wp, scp, G, idx_t, own_t, co_t, out_d, t,
               nb_floor=0.0, pair_floor=0.0):
    f32 = DT.float32
    tc.tile_set_cur_wait(nb_floor)

    # ---------------- neighbor stage ----------------
    own_b = _v(own_t, own_t[:, t, 0], [[0, K], [1, 3]])
    Gc = wp.tile([P, K, 3], f32, tag="Gc")
    nc.vector.tensor_tensor(out=Gc[:], in0=G[:], in1=own_b, op=ALU.subtract)

    # SmD stack [P, 5, 64]: planes fcn, invr, x, y, z; cols 32:64 duplicate 0:32
    SmD = wp.tile([P, 5, K2], f32, tag="SmD")
    gc_t = _v(Gc, Gc[:, 0, 0], [[1, 3], [3, K]])
    sm_xyz = _v(SmD, SmD[:, 2, 0], [[K2, 3], [1, K]])
    nc.vector.tensor_copy(out=sm_xyz, in_=gc_t)

    SQ = wp.tile([P, 3, K], f32, tag="SQ")
    nc.scalar.activation(SQ[:], gc_t, AF.Square)
    rsqD = wp.tile([P, K2], f32, tag="rsqD")
    sq_kc = _v(SQ, SQ[:, 0, 0], [[1, K], [K, 3]])
    nc.vector.tensor_reduce(
        out=rsqD[:, 0:K], in_=sq_kc, axis=mybir.AxisListType.X, op=ALU.add
    )

    # duplicate xyz+rsq columns immediately (they feed the critical
    # Mst-xyz -> dot -> qm chain); fc/inv planes duplicate after the fc chain
    nc.vector.tensor_copy(out=rsqD[:, K:K2], in_=rsqD[:, 0:K])
    nc.vector.tensor_copy(out=SmD[:, 2:5, K:K2], in_=SmD[:, 2:5, 0:K])

    r = wp.tile([P, K], f32, tag="r")
    nc.scalar.activation(r[:], rsqD[:, 0:K], AF.Sqrt)
    nc.vector.reciprocal(SmD[:, 1, 0:K], r[:])
    rm = wp.tile([P, K], f32, tag="rm")
    nc.vector.tensor_scalar_min(rm[:], r[:], RCUT)
    sn = wp.tile([P, K], f32, tag="sn")
    nc.scalar.activation(
        sn[:], rm[:], AF.Sin, bias=float(-np.pi / 2), scale=float(np.pi / RCUT)
    )
    nc.vector.tensor_scalar(SmD[:, 0, 0:K], sn[:], -0.5, 0.5, ALU.mult, ALU.add)
    nc.vector.tensor_copy(out=SmD[:, 0:2, K:K2], in_=SmD[:, 0:2, 0:K])

    # ---------------- G2 exponent stack (products emitted later) ----------------
    OUT51 = wp.tile([P, 51], f32, tag="OUT51")
    E2 = wp.tile([P, 8, K], f32, tag="E2")
    for i, eta in enumerate(G2_ETA):
        nc.scalar.activation(E2[:, i], rsqD[:, 0:K], AF.Exp, scale=-float(eta))

    tc.tile_set_cur_wait(pair_floor)

    # ---------------- pair stage [P, 512] ----------------
    # Mst planes: 0 fcj*fck, 1 invrj*invrk, 2..4 xx, yy, zz.
    # xyz planes first (they feed dot, the head of the critical chain); the
    # fc/inv planes are emitted below to fill the sqrt/sin ACT wait.
    Mst = wp.tile([P, 5, NP2], f32, tag="Mst")
    p_in0 = _v(SmD, SmD[:, 2, 0], [[K2, 3], [0, ND], [1, K]])
    p_in1 = _v(SmD, SmD[:, 2, 1], [[K2, 3], [1, ND], [1, K]])
    p_out = _v(Mst, Mst[:, 2, 0], [[NP2, 3], [K, ND], [1, K]])
    nc.vector.tensor_tensor(out=p_out, in0=p_in0, in1=p_in1, op=ALU.mult)

    SUMRSQ = wp.tile([P, NP2], f32, tag="SUMRSQ")
    s_in0 = _v(rsqD, rsqD[:, 0], [[0, ND], [1, K]])
    s_in1 = _v(rsqD, rsqD[:, 1], [[1, ND], [1, K]])
    s_out = _v(SUMRSQ, SUMRSQ[:, 0], [[K, ND], [1, K]])
    nc.vector.tensor_tensor(out=s_out, in0=s_in0, in1=s_in1, op=ALU.add)

    dot = wp.tile([P, NP2], f32, tag="dot")
    nc.vector.tensor_tensor(out=dot[:], in0=Mst[:, 2], in1=Mst[:, 3], op=ALU.add)
    nc.vector.tensor_tensor(out=dot[:], in0=dot[:], in1=Mst[:, 4], op=ALU.add)

    # qm = 2 dot - sumrsq = -rjk^2; clamp to [-64, 0]; rjk = sqrt(-qm)
    # (computed straight off dot so the sqrt/sin chain starts one hop sooner)
    qm = wp.tile([P, NP2], f32, tag="qm")
    nc.vector.scalar_tensor_tensor(
        out=qm[:], in0=dot[:], scalar=2.0, in1=SUMRSQ[:],
        op0=ALU.mult, op1=ALU.subtract,
    )
    qc = wp.tile([P, NP2], f32, tag="qc")
    nc.vector.tensor_scalar(qc[:], qm[:], 0.0, -64.0, ALU.min, ALU.max)
    rjk = wp.tile([P, NP2], f32, tag="rjk")
    nc.scalar.activation(rjk[:], qc[:], AF.Sqrt, scale=-1.0)
    csjk = wp.tile([P, NP2], f32, tag="csjk")       # cos(pi rjk / 8)
    nc.scalar.activation(
        csjk[:], rjk[:], AF.Sin, bias=float(np.pi / 2), scale=float(-np.pi / RCUT)
    )

    # fc/inv product planes (fills the sqrt/sin ACT wait on DVE)
    f_in0 = _v(SmD, SmD[:, 0, 0], [[K2, 2], [0, ND], [1, K]])
    f_in1 = _v(SmD, SmD[:, 0, 1], [[K2, 2], [1, ND], [1, K]])
    f_out = _v(Mst, Mst[:, 0, 0], [[NP2, 2], [K, ND], [1, K]])
    nc.vector.tensor_tensor(out=f_out, in0=f_in0, in1=f_in1, op=ALU.mult)

    sp_t = wp.tile([P, NP2], f32, tag="sp_t")       # s' = sumrsq - dot
    nc.vector.tensor_tensor(out=sp_t[:], in0=SUMRSQ[:], in1=dot[:], op=ALU.subtract)

    cos = wp.tile([P, NP2], f32, tag="cos")
    nc.vector.tensor_tensor(out=cos[:], in0=dot[:], in1=Mst[:, 1], op=ALU.mult)

    # ---------------- G2 (emitted here to fill the csjk wait) ----------------
    g2pr = wp.tile([P, 8, K], f32, tag="g2pr")
    fcn_b = _v(SmD, SmD[:, 0, 0], [[0, 8], [1, K]])
    nc.vector.tensor_tensor(out=g2pr[:], in0=E2[:], in1=fcn_b, op=ALU.mult)
    nc.vector.tensor_reduce(
        out=OUT51[:, 0:8], in_=g2pr[:], axis=mybir.AxisListType.X, op=ALU.add
    )

    w = wp.tile([P, NP2], f32, tag="w")             # fcj*fck*2fc_jk
    nc.vector.scalar_tensor_tensor(
        out=w[:], in0=csjk[:], scalar=1.0, in1=Mst[:, 0],
        op0=ALU.add, op1=ALU.mult,
    )
    # d=16 block double-counts each unordered pair: halve its weight
    nc.vector.tensor_scalar(
        w[:, NP2 - K : NP2], w[:, NP2 - K : NP2], 0.5, 0.0, ALU.mult, ALU.add
    )

    # ACT order: E0, E1 (ready off sp_t), c2, A-chain (to am4), E2..E6,
    # ap8/ap16 — interleaved so the DVE lane stream rarely waits on ACT.
    Es = []
    for e in range(7):
        Ee = wp.tile([P, NP2], f32, tag=f"Ee{e}", name=f"Ee{e}")
        Es.append(Ee)
    nc.scalar.activation(Es[0][:], sp_t[:], AF.Exp, scale=-2.0 * float(ETAS7[0]))
    nc.scalar.activation(Es[1][:], sp_t[:], AF.Exp, scale=-2.0 * float(ETAS7[1]))
    c2 = wp.tile([P, NP2], f32, tag="c2")
    nc.scalar.activation(c2[:], cos[:], AF.Square)
    ap1 = wp.tile([P, NP2], f32, tag="ap1")
    nc.scalar.activation(ap1[:], cos[:], AF.Identity, bias=0.5, scale=0.5)
    am1 = wp.tile([P, NP2], f32, tag="am1")
    nc.scalar.activation(am1[:], cos[:], AF.Identity, bias=0.5, scale=-0.5)
    ap2 = wp.tile([P, NP2], f32, tag="ap2")
    nc.scalar.activation(ap2[:], ap1[:], AF.Square)
    am2 = wp.tile([P, NP2], f32, tag="am2")
    nc.scalar.activation(am2[:], am1[:], AF.Square)
    ap4 = wp.tile([P, NP2], f32, tag="ap4")
    nc.scalar.activation(ap4[:], ap2[:], AF.Square)
    am4 = wp.tile([P, NP2], f32, tag="am4")
    nc.scalar.activation(am4[:], am2[:], AF.Square)
    for e in range(2, 7):
        nc.scalar.activation(Es[e][:], sp_t[:], AF.Exp, scale=-2.0 * float(ETAS7[e]))
    ap16 = wp.tile([P, NP2], f32, tag="ap16")
    nc.scalar.activation(ap16[:], ap4[:], AF.Square)      # ^8
    nc.scalar.activation(ap16[:], ap16[:], AF.Square)     # ^16

    # V bases (Pool for tile 1)
    V1 = wp.tile([P, NP2], f32, tag="V1")
    nc.vector.tensor_tensor(out=V1[:], in0=cos[:], in1=w[:], op=ALU.mult)
    V2 = wp.tile([P, NP2], f32, tag="V2")
    nc.vector.tensor_tensor(out=V2[:], in0=c2[:], in1=w[:], op=ALU.mult)
    Vm = [w, V1, V2]

    # ---------------- fused reduce lanes ----------------
    # lane list in emission order; the last POOL_LANES_PER_TILE[t] run as
    # Pool-mult + ACT-accumulate (both idle once gathers drain), the rest as
    # fused DVE scalar_tensor_tensor. u-base multiplies are emitted lazily
    # right before their first lane so the DVE stream doesn't stall on the
    # ACT square chain.
    Mbuf = wp.tile([P, NM, 8], f32, tag="Mbuf")
    u4m = wp.tile([P, NP2], f32, tag="u4m")
    u4p = wp.tile([P, NP2], f32, tag="u4p")
    u16 = wp.tile([P, NP2], f32, tag="u16")
    umult_of = {
        id(u4m): (u4m, am4),
        id(u4p): (u4p, ap4),
        id(u16): (u16, ap16),
    }
    # M-lanes (Mbuf targets) first, all on DVE, so the recombine can fire as
    # soon as they drain; the u4/u16 lanes (direct OUT51 columns) follow, with
    # the pool share taken from them so no Mbuf write trails on Pool/ACT.
    mlanes = []
    for e in range(7):
        for m in range(NM):
            mlanes.append((Vm[m], e, Mbuf[:, m, e : e + 1]))
    ulanes = []
    for e in range(7):
        ulanes.append((u4m, e, OUT51[:, 8 + 6 * e + 4 : 8 + 6 * e + 5]))
        ulanes.append((u4p, e, OUT51[:, 8 + 6 * e + 5 : 8 + 6 * e + 6]))
    ulanes.append((u16, 6, OUT51[:, 50:51]))

    n_pool = min(POOL_LANES_PER_TILE[t], len(ulanes))
    n_act = ACT_LANES_PER_TILE[t]
    NSCR = 4
    scrD = [
        scp.tile([P, NP2], f32, tag=f"scrD{i}", name=f"scrD{i}") for i in range(4)
    ]
    scrP = [
        scp.tile([P, NP2], f32, tag=f"scrP{i}", name=f"scrP{i}") for i in range(NSCR)
    ]
    scrA = [
        scp.tile([P, NP2], f32, tag=f"scrA{i}", name=f"scrA{i}") for i in range(NSCR)
    ]
    di = pi = ai = 0
    emitted_u = set()
    LM, LU = len(mlanes), len(ulanes)
    # keep the last two u-lanes (e6/z16) on fused DVE so trailing ACT
    # accumulates don't close the kernel
    LUp = max(LU - 2, 1)
    pool_idx = (
        {(2 * i + 1) * LUp // (2 * n_pool) for i in range(n_pool)} if n_pool else set()
    )
    # ACT-split lanes spread across the M-lane list
    act_idx = (
        {(2 * i + 1) * LM // (2 * n_act) for i in range(n_act)} if n_act else set()
    )
    # ACT accumulates are emitted LAGGED (2 behind their mult) so the in-order
    # ACT queue never dispatches an accum whose mult hasn't finished
    LAG = 2
    pending = []        # (scratch, acc)

    def drain_pending(limit):
        while len(pending) > limit:
            s, acc2 = pending.pop(0)
            nc.scalar.activation(s[:], s[:], AF.Identity, accum_out=acc2)

    def dve_lane(src, e, acc):
        # fused 1x: multiply + free-axis accumulate in one DVE op
        nonlocal di
        nc.vector.scalar_tensor_tensor(
            out=scrD[di % 4][:], in0=src[:], scalar=0.0, in1=Es[e][:],
            op0=ALU.bypass, op1=ALU.mult, accum_out=acc,
        )
        di += 1

    def act_lane(src, e, acc):
        # DVE 2x-mode mult (no accum) + lagged ACT accumulate
        nonlocal ai
        s = scrA[ai % NSCR]
        nc.vector.scalar_tensor_tensor(
            out=s[:], in0=src[:], scalar=0.0, in1=Es[e][:],
            op0=ALU.bypass, op1=ALU.mult,
        )
        pending.append((s, acc))
        ai += 1
        drain_pending(LAG)

    def pool_lane(src, e, acc):
        nonlocal pi
        s = scrP[pi % NSCR]
        nc.gpsimd.tensor_tensor(out=s[:], in0=src[:], in1=Es[e][:], op=ALU.mult)
        pending.append((s, acc))
        pi += 1
        drain_pending(LAG)

    def ensure_umult(src):
        if id(src) in umult_of and id(src) not in emitted_u:
            u_t, a_t = umult_of[id(src)]
            nc.vector.tensor_tensor(out=u_t[:], in0=a_t[:], in1=w[:], op=ALU.mult)
            emitted_u.add(id(src))

    # interleave: u-lanes are emitted among the M-lanes (2 M per 1 u) so both
    # engines start as soon as operands exist, but M-lanes stay off Pool
    ui = 0
    for li, (src, e, acc) in enumerate(mlanes):
        if li in act_idx:
            act_lane(src, e, acc)
        else:
            dve_lane(src, e, acc)
        while ui < LU and ui * LM < (li + 1) * LU:
            usrc, ue, uacc = ulanes[ui]
            ensure_umult(usrc)
            if ui in pool_idx:
                pool_lane(usrc, ue, uacc)
            else:
                dve_lane(usrc, ue, uacc)
            ui += 1
    drain_pending(0)

    # ---------------- recombine moments -> zeta<=2 columns ----------------
    PRc = wp.tile([P, NM, 28], f32, tag="PRc")
    co_v = _v(co_t, co_t[:, 0, 0], [[4, NM], [0, 7], [1, 4]])
    m_v = _v(Mbuf, Mbuf[:, 0, 0], [[8, NM], [1, 7], [0, 4]])
    pr_v = _v(PRc, PRc[:, 0, 0], [[28, NM], [4, 7], [1, 4]])
    nc.vector.tensor_tensor(out=pr_v, in0=co_v, in1=m_v, op=ALU.mult)
    R28 = wp.tile([P, 28], f32, tag="R28")
    red_v = _v(PRc, PRc[:, 0, 0], [[1, 28], [28, NM]])
    nc.vector.tensor_reduce(
        out=R28[:], in_=red_v, axis=mybir.AxisListType.X, op=ALU.add
    )
    out28 = _v(OUT51, OUT51[:, 8], [[6, 7], [1, 4]])
    nc.vector.tensor_copy(out=out28, in_=R28[:])

    nc.sync.dma_start(out=out_d[:, t], in_=OUT51[:])


_NC_CACHE = None


def _get_nc():
    global _NC_CACHE
    if _NC_CACHE is None:
        _NC_CACHE = build_nc()
    return _NC_CACHE


def make_inputs(pos, numnei, neighs):
    """Host-side shard prep: per-core idx/own in device layout."""
    pos = np.asarray(pos, np.float32)
    numnei = np.asarray(numnei, np.int32)
    neighs = np.asarray(neighs, np.int32)
    idx = neighs.reshape(N_ATOMS, K).copy()
    kk = np.arange(K)[None, :]
    idx[kk >= numnei[:, None]] = SENT
    tbl = np.concatenate([pos, np.full((1, 3), 1.0e4, np.float32)], axis=0)

    co = np.zeros((P, NM, 4), np.float32)
    for ci in range(4):
        for m in range(NM):
            co[:, m, ci] = CO4[ci][m]

    in_maps = []
    for c in range(N_CORES):
        idxd = np.full((P, TILES, K), SENT, np.int32)
        ownd = np.zeros((P, TILES, 3), np.float32)
        for t in range(TILES):
            g0 = c * APC + t * P
            n = min(P, APC - t * P)
            if n <= 0:
                continue
            idxd[:n, t] = idx[g0 : g0 + n]
            ownd[:n, t] = pos[g0 : g0 + n]
        in_maps.append({"pos_tbl": tbl, "idx": idxd, "own": ownd, "co": co})
    return in_maps


def unshard_output(results):
    out = np.empty((N_ATOMS, 51), np.float32)
    for c in range(N_CORES):
        o = results[c]["out"]            # [P, TILES, 51]
        for t in range(TILES):
            g0 = c * APC + t * P
            n = min(P, APC - t * P)
            if n <= 0:
                continue
            out[g0 : g0 + n] = o[:n, t]
    return out


def run(pos, numnei, neighs, trace=False):
    nc = _get_nc()
    in_maps = make_inputs(pos, numnei, neighs)
    res = run_bass_kernel_spmd(nc, in_maps, list(range(N_CORES)), trace=trace)
    return unshard_output(res.results), res


def kernel(pos, numnei, neighs):
    out, _ = run(pos, numnei, neighs)
    return out
